# revision 19
# baseline (speedup 1.0000x reference)
"""Trainium2 Bass kernel for nn_MultiHeadedAttentionWithGate (v2).

Math (per molecule, validated against reference):
  The reference's reshapes are flat views, so with u = "virtual row"
  (1024 per molecule), the computation is per-u over contiguous flat
  segments: K/V/M rows of 320 (10 nei x 32), X rows of 640 (10 x 64),
  q rows of 32.  u = 4*(128*G + p) + r with p on partitions, G in 0..1,
  phase r in 0..3.  For fixed r the K/V/M 320-segments decompose into
  two row-block pieces d = r (cols 64r..256) and d = r+1 (cols
  0..64+64r), where row-block d of the projections is X^T chunk-matmuls
  against W rows.

v2 changes vs baseline:
  - PE transposes + PSUM->SBUF copies replaced by XBAR DMA transpose
    (one [128,2560]->[128,20,128] instruction per G-block).
  - K and Wam projections run in fp8e4m3 DoubleRow mode (2x PE): X^T is
    cast f16->fp8 once per block; W scaled by 64 into fp8 range, with
    1/64 folded into Wq (score path) and wg_emax (gate path).
  - Separate K/V/M PSUM pools (2/3/2 banks) so each engine-side consumer
    releases its bank independently; V triple-buffered because its
    consumer (the softmax-weighted product) sits deepest in the chain.
  - Elementwise ops in f16; engine assignment spread over
    gpsimd/vector/scalar; X loaded in one converting DMA per G-block on
    the sync HWDGE queue.

Sharding: data-parallel over batch: 8 molecules per core x 8 cores.
"""

import sys

for _p in ("/opt/trn_rl_repo", "/root/.axon_site/_ro/trn_rl_repo"):
    if _p not in sys.path:
        sys.path.insert(0, _p)

from contextlib import ExitStack

import numpy as np

import concourse.bass as bass
import concourse.mybir as mybir
from concourse import bacc
from concourse.tile import TileContext

F8 = mybir.dt.float8e4
F16 = mybir.dt.float16
F32 = mybir.dt.float32
EXP = mybir.ActivationFunctionType.Exp
ADD = mybir.AluOpType.add
MAX = mybir.AluOpType.max
AXL_X = mybir.AxisListType.X
DR = mybir.MatmulPerfMode.DoubleRow

N_CORES = 8
BM = 8          # molecules per core
A = 128         # atoms
NEI = 10
D = 256
D2 = 512

USE_FP8_K = False   # fp8 K scores cost ~2.9e-2 rel err (gate is 2e-2)
USE_FP8_M = True
WSCALE = 64.0


def build_nc(with_bias: bool, bg_val: float, bm: int = BM) -> bass.Bass:
    nc = bacc.Bacc("TRN2", target_bir_lowering=False)

    x_h = nc.declare_dram_parameter("x", [bm, A * NEI, D2], F32, isOutput=False)
    qin_h = nc.declare_dram_parameter("qin", [bm, A, D], F32, isOutput=False)
    wv_h = nc.declare_dram_parameter("wv", [128, 4, 256], F16, isOutput=False)
    wq_h = nc.declare_dram_parameter("wq", [128, 2, 256], F16, isOutput=False)
    if USE_FP8_K:
        w8k_h = nc.declare_dram_parameter("w8k", [128, 2, 2, 256], F8,
                                          isOutput=False)
    else:
        w16k_h = nc.declare_dram_parameter("w16k", [128, 4, 256], F16,
                                           isOutput=False)
    if USE_FP8_M:
        w8m_h = nc.declare_dram_parameter("w8m", [128, 2, 2, 256], F8,
                                          isOutput=False)
    else:
        w16m_h = nc.declare_dram_parameter("w16m", [128, 4, 256], F16,
                                           isOutput=False)
    ssel_h = nc.declare_dram_parameter("ssel", [128, 32], F16, isOutput=False)
    s2sel_h = nc.declare_dram_parameter("s2sel", [32, 128], F16, isOutput=False)
    wgc_h = nc.declare_dram_parameter("wg_cur", [128, 32], F32, isOutput=False)
    wge_h = nc.declare_dram_parameter("wg_emax", [128, 32], F16, isOutput=False)
    wgav_h = nc.declare_dram_parameter("wg_avc", [128, 5], F16, isOutput=False)
    if with_bias:
        bcat_h = nc.declare_dram_parameter("bcat", [1, 3, 256], F16,
                                           isOutput=False)
        bq_h = nc.declare_dram_parameter("bq", [1, 256], F16, isOutput=False)
        ones_h = nc.declare_dram_parameter("ones", [1, 128], F16,
                                           isOutput=False)
    out_h = nc.declare_dram_parameter("out", [bm, A, D], F32, isOutput=True)

    # flat per-molecule views
    xg = (x_h[:].rearrange("b n c -> b (n c)")
          .rearrange("b (g p t) -> b g p t", g=2, p=128, t=2560))
    q5 = (qin_h[:].rearrange("b a c -> b (a c)")
          .rearrange("b (g p r k) -> b g p r k", g=2, p=128, r=4, k=32))
    o5 = (out_h[:].rearrange("b a c -> b (a c)")
          .rearrange("b (g p r k) -> b g p r k", g=2, p=128, r=4, k=32))

    with TileContext(nc) as tc, ExitStack() as ctx:
        ctx.enter_context(nc.allow_low_precision(
            reason="f16 intermediates validated against the fp32 reference"))
        consts = ctx.enter_context(tc.tile_pool(name="consts", bufs=1))
        sb_xu = ctx.enter_context(tc.tile_pool(name="xu", bufs=3))
        sb_xtf = ctx.enter_context(tc.tile_pool(name="xtf", bufs=2))
        sb_xt8 = ctx.enter_context(tc.tile_pool(name="xt8", bufs=2))
        sb_ew = ctx.enter_context(tc.tile_pool(name="ew", bufs=2))
        sb_stash = ctx.enter_context(tc.tile_pool(name="stash", bufs=2))
        sb_q = ctx.enter_context(tc.tile_pool(name="qp", bufs=3))
        ps_k = ctx.enter_context(tc.tile_pool(name="pk", bufs=2, space="PSUM"))
        ps_v = ctx.enter_context(tc.tile_pool(name="pv", bufs=2, space="PSUM"))
        ps_m = ctx.enter_context(tc.tile_pool(name="pmx", bufs=2, space="PSUM"))
        ps_misc = ctx.enter_context(tc.tile_pool(name="pm", bufs=2,
                                                 space="PSUM"))
        dram = ctx.enter_context(tc.tile_pool(name="dram", bufs=1,
                                              space="DRAM"))

        def cload(h, shape, dtype):
            t = consts.tile(shape, dtype, tag=h.name)
            nc.sync.dma_start(out=t, in_=h[:])
            return t

        wv_t = cload(wv_h, [128, 4, 256], F16)
        wq_t = cload(wq_h, [128, 2, 256], F16)
        if USE_FP8_K:
            w8k_t = cload(w8k_h, [128, 2, 2, 256], F8)
        else:
            w16k_t = cload(w16k_h, [128, 4, 256], F16)
        if USE_FP8_M:
            w8m_t = cload(w8m_h, [128, 2, 2, 256], F8)
        else:
            w16m_t = cload(w16m_h, [128, 4, 256], F16)
        ssel_t = cload(ssel_h, [128, 32], F16)
        s2sel_t = cload(s2sel_h, [32, 128], F16)
        wgc_t = cload(wgc_h, [128, 32], F32)
        wge_t = cload(wge_h, [128, 32], F16)
        wgav_t = cload(wgav_h, [128, 5], F16)
        if with_bias:
            bcat_t = cload(bcat_h, [1, 3, 256], F16)
            bq_t = cload(bq_h, [1, 256], F16)
            ones_t = cload(ones_h, [1, 128], F16)

        qdram = dram.tile([bm, A * D], F16)

        # ---- all q projections up front (natural layout) -> DRAM ----
        # q is scaled by 1/WSCALE (folded into wq) iff K runs in fp8.
        for mol in range(bm):
            qin16 = sb_q.tile([128, 256], F16, tag="qin16")
            nc.gpsimd.dma_start(out=qin16, in_=qin_h[mol])
            qT = sb_q.tile([128, 2, 128], F16, tag="qT")
            nc.scalar.dma_start_transpose(out=qT, in_=qin16)
            qpsum = ps_misc.tile([128, 256], F32, tag="pm", name="qpsum")
            nc.tensor.matmul(qpsum, qT[:, 0, :], wq_t[:, 0, :],
                             start=True, stop=False)
            nc.tensor.matmul(qpsum, qT[:, 1, :], wq_t[:, 1, :],
                             start=False, stop=not with_bias)
            if with_bias:
                nc.tensor.matmul(qpsum, ones_t, bq_t, start=False, stop=True)
            qnat = sb_q.tile([128, 256], F16, tag="qnat")
            nc.vector.tensor_copy(out=qnat, in_=qpsum)
            nc.scalar.dma_start(
                out=qdram[mol].rearrange("(a c) -> a c", a=128), in_=qnat)

        qu_view = qdram[:].rearrange(
            "b (g p r k) -> b g p r k", g=2, p=128, r=4, k=32)

        for mol in range(bm):
            for G in range(2):
                # ---- X load (one converting DMA) + XBAR transpose ----
                xu16 = sb_xu.tile([128, 2560], F16, tag="xu")
                nc.gpsimd.dma_start(out=xu16, in_=xg[mol, G])
                xtf = sb_xtf.tile([128, 20, 128], F16, tag="xtf")
                nc.sync.dma_start_transpose(out=xtf, in_=xu16)
                if USE_FP8_K or USE_FP8_M:
                    xt8 = sb_xt8.tile([128, 20, 128], F8, tag="xt8")
                    nc.scalar.copy(out=xt8[:, 0:10, :], in_=xtf[:, 0:10, :])
                    nc.gpsimd.tensor_copy(out=xt8[:, 10:20, :],
                                          in_=xtf[:, 10:20, :])

                cur4 = sb_ew.tile([128, 4, 32], F32, tag="cur4")
                nc.sync.dma_start(out=cur4, in_=q5[mol, G])
                qu4 = sb_ew.tile([128, 4, 32], F16, tag="qu4")
                nc.sync.dma_start(out=qu4, in_=qu_view[mol, G])

                smulB = sb_ew.tile([128, 4, 320], F16, tag="smulB")
                v16B = sb_ew.tile([128, 4, 320], F16, tag="v16B")
                amulB = sb_ew.tile([128, 4, 32, 10], F16, tag="amulB")
                exB = sb_ew.tile([128, 4, 10], F16, tag="exB")
                scoreB = sb_ew.tile([128, 4, 10], F16, tag="scoreB")
                emaxB = sb_ew.tile([128, 4, 32], F16, tag="emaxB")
                adenB = sb_ew.tile([128, 4], F32, tag="adenB")
                arawB = sb_stash.tile([128, 4, 32], F16, tag="arawB")
                gaveB = sb_ew.tile([128, 4], F32, tag="gaveB")
                pg = ps_misc.tile([128, 4], F32, tag="pm", name="pg")

                for r in range(4):
                    wA = 256 - 64 * r
                    ranges = [(r, 0, wA, 64 * r), (r + 1, wA, 320 - wA, 0)]
                    kp = ps_k.tile([128, 320], F32, tag="pk", name=f"kp{r}",
                                   padded_shape=[128, 512])
                    vp = ps_v.tile([128, 320], F32, tag="pv", name=f"vp{r}",
                                   padded_shape=[128, 512])
                    mp = ps_m.tile([128, 320], F32, tag="pmx", name=f"mp{r}",
                                   padded_shape=[128, 512])
                    for (d, t0, wd, e0) in ranges:
                        # K and M (fp8 DoubleRow) with stationary xt8 pairs.
                        # K's start=True must be the first write into the kp
                        # bank: it marks the whole 2KB bank pending-zero,
                        # which also zero-initializes pg's column (pg then
                        # accumulates with start=False throughout).
                        for pair in range(2):
                            c0 = 4 * d + 2 * pair
                            if USE_FP8_K:
                                nc.tensor.matmul(
                                    kp[:, t0:t0 + wd],
                                    xt8[:, c0:c0 + 2, :],
                                    w8k_t[:, pair, :, e0:e0 + wd],
                                    start=(d == r and pair == 0),
                                    stop=(d == r + 1 and pair == 1
                                          and not with_bias),
                                    perf_mode=DR, skip_group_check=True)
                            if USE_FP8_M:
                                nc.tensor.matmul(
                                    mp[:, t0:t0 + wd],
                                    xt8[:, c0:c0 + 2, :],
                                    w8m_t[:, pair, :, e0:e0 + wd],
                                    start=(d == r and pair == 0),
                                    stop=(d == r + 1 and pair == 1
                                          and not with_bias),
                                    perf_mode=DR, skip_group_check=True)
                        if not USE_FP8_K or not USE_FP8_M:
                            for fc in range(4):
                                ch = 4 * d + fc
                                if not USE_FP8_K:
                                    nc.tensor.matmul(
                                        kp[:, t0:t0 + wd], xtf[:, ch, :],
                                        w16k_t[:, fc, e0:e0 + wd],
                                        start=(d == r and fc == 0),
                                        stop=(d == r + 1 and fc == 3
                                              and not with_bias),
                                        skip_group_check=True)
                                if not USE_FP8_M:
                                    nc.tensor.matmul(
                                        mp[:, t0:t0 + wd], xtf[:, ch, :],
                                        w16m_t[:, fc, e0:e0 + wd],
                                        start=(d == r and fc == 0),
                                        stop=(d == r + 1 and fc == 3
                                              and not with_bias),
                                        skip_group_check=True)
                        # V (f16) with stationary xtf chunks; pg rides along
                        # (start=False: K's bank-wide start zero-inits pg)
                        for fc in range(4):
                            ch = 4 * d + fc
                            nc.tensor.matmul(
                                vp[:, t0:t0 + wd], xtf[:, ch, :],
                                wv_t[:, fc, e0:e0 + wd],
                                start=(d == r and fc == 0),
                                stop=(d == r + 1 and fc == 3
                                      and not with_bias),
                                skip_group_check=True)
                            w_pg = ch - 5 * r
                            if 0 <= w_pg < 5:
                                nc.tensor.matmul(
                                    pg[:, r:r + 1], xtf[:, ch, :],
                                    wgav_t[:, w_pg:w_pg + 1],
                                    start=(w_pg == 0), stop=(w_pg == 4),
                                    skip_group_check=True)
                        if with_bias:
                            for ps, i in ((kp, 0), (vp, 1), (mp, 2)):
                                nc.tensor.matmul(
                                    ps[:, t0:t0 + wd], ones_t,
                                    bcat_t[:, i, e0:e0 + wd],
                                    start=False, stop=(d == r + 1),
                                    skip_group_check=True)

                    # ---- per-phase PSUM consumers (free the banks) ----
                    # V -> SBUF f16 on scalar (gpsimd cannot read PSUM)
                    nc.scalar.copy(out=v16B[:, r, :], in_=vp[:, 0:320])
                    # score product on vector (PSUM-sourced)
                    nc.vector.tensor_mul(
                        smulB[:, r, :], kp[:, 0:320],
                        qu4[:, r, :].unsqueeze(1).broadcast_to([128, 10, 32]))
                    nc.vector.tensor_reduce(
                        out=emaxB[:, r, :],
                        in_=mp[:, 0:320].rearrange("p (j k) -> p k j", j=10),
                        axis=AXL_X, op=MAX)

                # ---- batched per-G math (all SBUF f16) ----
                nc.vector.tensor_reduce(
                    out=scoreB,
                    in_=smulB.rearrange("p r (j k) -> p r j k", j=10),
                    axis=AXL_X, op=ADD)
                nc.scalar.activation(out=exB, in_=scoreB, func=EXP)
                nc.vector.tensor_reduce(out=adenB, in_=exB, axis=AXL_X, op=ADD)
                # attention numerator, written k-major for a contiguous
                # inner reduce
                nc.gpsimd.tensor_mul(
                    amulB.rearrange("p r k j -> p r j k"),
                    v16B.rearrange("p r (j k) -> p r j k", j=10),
                    exB.unsqueeze(3).broadcast_to([128, 4, 10, 32]))
                nc.vector.tensor_reduce(
                    out=arawB, in_=amulB, axis=AXL_X, op=ADD)
                nc.vector.tensor_copy(out=gaveB, in_=pg)
                curp = sb_ew.tile([128, 4, 32], F32, tag="curp")
                nc.gpsimd.tensor_mul(
                    curp, cur4,
                    wgc_t.unsqueeze(1).broadcast_to([128, 4, 32]))
                gcurB = sb_ew.tile([128, 4], F32, tag="gcurB")
                nc.vector.tensor_reduce(out=gcurB, in_=curp, axis=AXL_X,
                                        op=ADD)
                emaxp = sb_ew.tile([128, 4, 32], F16, tag="emaxp")
                nc.gpsimd.tensor_mul(
                    emaxp, emaxB,
                    wge_t.unsqueeze(1).broadcast_to([128, 4, 32]))
                gemxB = sb_ew.tile([128, 4], F32, tag="gemxB")
                nc.vector.tensor_reduce(out=gemxB, in_=emaxp, axis=AXL_X,
                                        op=ADD)
                gl1 = sb_ew.tile([128, 4], F32, tag="gl1")
                nc.vector.tensor_add(gl1, gcurB, gemxB)
                gl2 = sb_ew.tile([128, 4], F32, tag="gl2")
                nc.vector.tensor_add(gl2, gl1, gaveB)
                egB = sb_stash.tile([128, 4], F32, tag="egB")
                nc.scalar.activation(out=egB, in_=gl2, func=EXP,
                                     bias=float(bg_val))
                egB16 = sb_stash.tile([128, 4], F16, tag="egB16")
                nc.vector.tensor_copy(out=egB16, in_=egB)
                raB = sb_stash.tile([128, 4], F32, tag="raB")
                nc.vector.reciprocal(out=raB, in_=adenB)

                if G == 0:
                    st0 = (arawB, egB, egB16, raB)
                else:
                    gd = ps_misc.tile([32, 4], F32, tag="pm", name="gd")
                    for r in range(4):
                        nc.tensor.matmul(gd[:, r:r + 1], ssel_t,
                                         st0[2][:, r:r + 1],
                                         start=True, stop=False)
                        nc.tensor.matmul(gd[:, r:r + 1], ssel_t,
                                         egB16[:, r:r + 1],
                                         start=False, stop=True)
                    rg = sb_ew.tile([32, 4], F32, tag="rg")
                    nc.vector.reciprocal(out=rg, in_=gd)
                    rg16 = sb_ew.tile([32, 4], F16, tag="rg16")
                    nc.vector.tensor_copy(out=rg16, in_=rg)
                    inv = ps_misc.tile([128, 4], F32, tag="pm", name="inv")
                    for r in range(4):
                        nc.tensor.matmul(inv[:, r:r + 1], s2sel_t,
                                         rg16[:, r:r + 1],
                                         start=True, stop=True)
                    c2B = {}
                    for gg, (ar_g, eg_g, eg16_g, ra_g) in (
                            (0, st0), (1, (arawB, egB, egB16, raB))):
                        t1 = sb_ew.tile([128, 4], F32, tag="t1", name="t1")
                        nc.vector.tensor_mul(t1, inv, ra_g)
                        c2B[gg] = sb_stash.tile([128, 4], F32, tag=f"c2B{gg}",
                                                name=f"c2B{gg}")
                        nc.vector.tensor_mul(c2B[gg], t1, eg_g)
                    for gg, ar_g in ((0, st0[0]), (1, arawB)):
                        outB = sb_ew.tile([128, 4, 32], F32, tag="outB")
                        nc.gpsimd.tensor_mul(
                            outB, ar_g,
                            c2B[gg].unsqueeze(2).broadcast_to([128, 4, 32]))
                        nc.scalar.dma_start(out=o5[mol, gg], in_=outB)
    nc.finalize()
    return nc


def _q8(w):
    import ml_dtypes
    return np.asarray(w, np.float32).astype(ml_dtypes.float8_e4m3fn)


def _prep_consts(Wq, bq, Wk, bk, Wv, bv, Wam, bam, Wg, bg):
    wv = np.empty((128, 4, 256), np.float16)
    for fc in range(4):
        wv[:, fc, :] = Wv[128 * fc:128 * (fc + 1), :]
    consts = {"wv": wv}
    kscale = WSCALE if USE_FP8_K else 1.0
    mscale = WSCALE if USE_FP8_M else 1.0
    if USE_FP8_K:
        w8k = np.empty((128, 2, 2, 256), np.float32)
        for pair in range(2):
            for i in range(2):
                fc = 2 * pair + i
                w8k[:, pair, i, :] = Wk[128 * fc:128 * (fc + 1), :] * kscale
        consts["w8k"] = _q8(w8k)
    else:
        w16k = np.empty((128, 4, 256), np.float16)
        for fc in range(4):
            w16k[:, fc, :] = Wk[128 * fc:128 * (fc + 1), :]
        consts["w16k"] = w16k
    if USE_FP8_M:
        w8m = np.empty((128, 2, 2, 256), np.float32)
        for pair in range(2):
            for i in range(2):
                fc = 2 * pair + i
                w8m[:, pair, i, :] = Wam[128 * fc:128 * (fc + 1), :] * mscale
        consts["w8m"] = _q8(w8m)
    else:
        w16m = np.empty((128, 4, 256), np.float16)
        for fc in range(4):
            w16m[:, fc, :] = Wam[128 * fc:128 * (fc + 1), :]
        consts["w16m"] = w16m
    wqs = np.empty((128, 2, 256), np.float16)
    for fc in range(2):
        wqs[:, fc, :] = Wq[128 * fc:128 * (fc + 1), :] / kscale
    consts["wq"] = wqs
    p = np.arange(128)
    ssel = (p[:, None] % 32 == np.arange(32)[None, :]).astype(np.float16)
    consts["ssel"] = ssel
    consts["s2sel"] = ssel.T.copy()
    wg = np.asarray(Wg[:, 0], np.float32)
    consts["wg_cur"] = np.tile(wg[0:32], (128, 1)).astype(np.float32)
    consts["wg_emax"] = np.tile(wg[32:64] / mscale, (128, 1)).astype(np.float16)
    wgav = np.empty((128, 5), np.float32)
    for w in range(5):
        wgav[:, w] = wg[64 + (np.arange(128) % 64)] / NEI
    consts["wg_avc"] = wgav.astype(np.float16)
    with_bias = any(np.any(np.asarray(b) != 0) for b in (bq, bk, bv, bam))
    if with_bias:
        bcat = np.stack([np.asarray(bk) * kscale, np.asarray(bv),
                         np.asarray(bam) * mscale]
                        ).astype(np.float16)[None, :, :].reshape(1, 3, 256)
        consts["bcat"] = bcat
        consts["bq"] = (np.asarray(bq) / kscale).astype(np.float16).reshape(
            1, 256)
        consts["ones"] = np.ones((1, 128), np.float16)
    return consts, with_bias, float(np.asarray(bg).reshape(-1)[0])


_CACHE = {}
TRACE = False       # set by test.py for profiling runs
LAST_RESULTS = None  # BassKernelResults from the most recent run


def kernel(input_multihead, input_q, Wq, bq, Wk, bk, Wv, bv, Wam, bam, Wg, bg):
    from concourse.bass_utils import run_bass_kernel_spmd

    consts, with_bias, bg_val = _prep_consts(
        Wq, bq, Wk, bk, Wv, bv, Wam, bam, Wg, bg)

    key = (with_bias, bg_val)
    if key not in _CACHE:
        _CACHE[key] = build_nc(with_bias, bg_val)
    nc = _CACHE[key]

    x = np.ascontiguousarray(np.asarray(input_multihead, np.float32))
    q = np.ascontiguousarray(np.asarray(input_q, np.float32))
    in_maps = []
    for c in range(N_CORES):
        m = {"x": x[BM * c:BM * (c + 1)], "qin": q[BM * c:BM * (c + 1)]}
        m.update(consts)
        in_maps.append(m)

    res = run_bass_kernel_spmd(nc, in_maps, list(range(N_CORES)), trace=TRACE)
    global LAST_RESULTS
    LAST_RESULTS = res
    return np.concatenate([res.results[c]["out"] for c in range(N_CORES)],
                          axis=0)


# revision 20
# speedup vs baseline: 1.0977x; 1.0977x over previous
"""Trainium2 Bass kernel for nn_MultiHeadedAttentionWithGate (v2).

Math (per molecule, validated against reference):
  The reference's reshapes are flat views, so with u = "virtual row"
  (1024 per molecule), the computation is per-u over contiguous flat
  segments: K/V/M rows of 320 (10 nei x 32), X rows of 640 (10 x 64),
  q rows of 32.  u = 4*(128*G + p) + r with p on partitions, G in 0..1,
  phase r in 0..3.  For fixed r the K/V/M 320-segments decompose into
  two row-block pieces d = r (cols 64r..256) and d = r+1 (cols
  0..64+64r), where row-block d of the projections is X^T chunk-matmuls
  against W rows.

v2 changes vs baseline:
  - PE transposes + PSUM->SBUF copies replaced by XBAR DMA transpose
    (one [128,2560]->[128,20,128] instruction per G-block).
  - K and Wam projections run in fp8e4m3 DoubleRow mode (2x PE): X^T is
    cast f16->fp8 once per block; W scaled by 64 into fp8 range, with
    1/64 folded into Wq (score path) and wg_emax (gate path).
  - Separate K/V/M PSUM pools (2/3/2 banks) so each engine-side consumer
    releases its bank independently; V triple-buffered because its
    consumer (the softmax-weighted product) sits deepest in the chain.
  - Elementwise ops in f16; engine assignment spread over
    gpsimd/vector/scalar; X loaded in one converting DMA per G-block on
    the sync HWDGE queue.

Sharding: data-parallel over batch: 8 molecules per core x 8 cores.
"""

import sys

for _p in ("/opt/trn_rl_repo", "/root/.axon_site/_ro/trn_rl_repo"):
    if _p not in sys.path:
        sys.path.insert(0, _p)

from contextlib import ExitStack

import numpy as np

import concourse.bass as bass
import concourse.mybir as mybir
from concourse import bacc
from concourse.tile import TileContext

F8 = mybir.dt.float8e4
F16 = mybir.dt.float16
F32 = mybir.dt.float32
EXP = mybir.ActivationFunctionType.Exp
ADD = mybir.AluOpType.add
MAX = mybir.AluOpType.max
AXL_X = mybir.AxisListType.X
DR = mybir.MatmulPerfMode.DoubleRow

N_CORES = 8
BM = 8          # molecules per core
A = 128         # atoms
NEI = 10
D = 256
D2 = 512

USE_FP8_K = False   # fp8 K scores cost ~2.9e-2 rel err (gate is 2e-2)
USE_FP8_M = False
WSCALE = 64.0


def build_nc(with_bias: bool, bg_val: float, bm: int = BM) -> bass.Bass:
    nc = bacc.Bacc("TRN2", target_bir_lowering=False)

    x_h = nc.declare_dram_parameter("x", [bm, A * NEI, D2], F32, isOutput=False)
    qin_h = nc.declare_dram_parameter("qin", [bm, A, D], F32, isOutput=False)
    wv_h = nc.declare_dram_parameter("wv", [128, 4, 256], F16, isOutput=False)
    wq_h = nc.declare_dram_parameter("wq", [128, 2, 256], F16, isOutput=False)
    if USE_FP8_K:
        w8k_h = nc.declare_dram_parameter("w8k", [128, 2, 2, 256], F8,
                                          isOutput=False)
    else:
        w16k_h = nc.declare_dram_parameter("w16k", [128, 4, 256], F16,
                                           isOutput=False)
    if USE_FP8_M:
        w8m_h = nc.declare_dram_parameter("w8m", [128, 2, 2, 256], F8,
                                          isOutput=False)
    else:
        w16m_h = nc.declare_dram_parameter("w16m", [128, 4, 256], F16,
                                           isOutput=False)
    ssel_h = nc.declare_dram_parameter("ssel", [128, 32], F16, isOutput=False)
    s2sel_h = nc.declare_dram_parameter("s2sel", [32, 128], F16, isOutput=False)
    wgc_h = nc.declare_dram_parameter("wg_cur", [128, 32], F32, isOutput=False)
    wge_h = nc.declare_dram_parameter("wg_emax", [128, 32], F16, isOutput=False)
    wgav_h = nc.declare_dram_parameter("wg_avc", [128, 5], F16, isOutput=False)
    if with_bias:
        bcat_h = nc.declare_dram_parameter("bcat", [1, 3, 256], F16,
                                           isOutput=False)
        bq_h = nc.declare_dram_parameter("bq", [1, 256], F16, isOutput=False)
        ones_h = nc.declare_dram_parameter("ones", [1, 128], F16,
                                           isOutput=False)
    out_h = nc.declare_dram_parameter("out", [bm, A, D], F32, isOutput=True)

    # flat per-molecule views
    xg = (x_h[:].rearrange("b n c -> b (n c)")
          .rearrange("b (g p t) -> b g p t", g=2, p=128, t=2560))
    q5 = (qin_h[:].rearrange("b a c -> b (a c)")
          .rearrange("b (g p r k) -> b g p r k", g=2, p=128, r=4, k=32))
    o5 = (out_h[:].rearrange("b a c -> b (a c)")
          .rearrange("b (g p r k) -> b g p r k", g=2, p=128, r=4, k=32))

    with TileContext(nc) as tc, ExitStack() as ctx:
        ctx.enter_context(nc.allow_low_precision(
            reason="f16 intermediates validated against the fp32 reference"))
        consts = ctx.enter_context(tc.tile_pool(name="consts", bufs=1))
        sb_xu = ctx.enter_context(tc.tile_pool(name="xu", bufs=3))
        sb_xtf = ctx.enter_context(tc.tile_pool(name="xtf", bufs=2))
        sb_xt8 = ctx.enter_context(tc.tile_pool(name="xt8", bufs=2))
        sb_ew = ctx.enter_context(tc.tile_pool(name="ew", bufs=2))
        sb_stash = ctx.enter_context(tc.tile_pool(name="stash", bufs=2))
        sb_q = ctx.enter_context(tc.tile_pool(name="qp", bufs=3))
        ps_k = ctx.enter_context(tc.tile_pool(name="pk", bufs=2, space="PSUM"))
        ps_v = ctx.enter_context(tc.tile_pool(name="pv", bufs=2, space="PSUM"))
        ps_m = ctx.enter_context(tc.tile_pool(name="pmx", bufs=2, space="PSUM"))
        ps_misc = ctx.enter_context(tc.tile_pool(name="pm", bufs=2,
                                                 space="PSUM"))
        dram = ctx.enter_context(tc.tile_pool(name="dram", bufs=1,
                                              space="DRAM"))

        def cload(h, shape, dtype):
            t = consts.tile(shape, dtype, tag=h.name)
            nc.sync.dma_start(out=t, in_=h[:])
            return t

        wv_t = cload(wv_h, [128, 4, 256], F16)
        wq_t = cload(wq_h, [128, 2, 256], F16)
        if USE_FP8_K:
            w8k_t = cload(w8k_h, [128, 2, 2, 256], F8)
        else:
            w16k_t = cload(w16k_h, [128, 4, 256], F16)
        if USE_FP8_M:
            w8m_t = cload(w8m_h, [128, 2, 2, 256], F8)
        else:
            w16m_t = cload(w16m_h, [128, 4, 256], F16)
        ssel_t = cload(ssel_h, [128, 32], F16)
        s2sel_t = cload(s2sel_h, [32, 128], F16)
        wgc_t = cload(wgc_h, [128, 32], F32)
        wge_t = cload(wge_h, [128, 32], F16)
        wgav_t = cload(wgav_h, [128, 5], F16)
        if with_bias:
            bcat_t = cload(bcat_h, [1, 3, 256], F16)
            bq_t = cload(bq_h, [1, 256], F16)
            ones_t = cload(ones_h, [1, 128], F16)

        qdram = dram.tile([bm, A * D], F16)

        # ---- all q projections up front (natural layout) -> DRAM ----
        # q is scaled by 1/WSCALE (folded into wq) iff K runs in fp8.
        for mol in range(bm):
            qin16 = sb_q.tile([128, 256], F16, tag="qin16")
            nc.gpsimd.dma_start(out=qin16, in_=qin_h[mol])
            qT = sb_q.tile([128, 2, 128], F16, tag="qT")
            nc.scalar.dma_start_transpose(out=qT, in_=qin16)
            qpsum = ps_misc.tile([128, 256], F32, tag="pm", name="qpsum")
            nc.tensor.matmul(qpsum, qT[:, 0, :], wq_t[:, 0, :],
                             start=True, stop=False)
            nc.tensor.matmul(qpsum, qT[:, 1, :], wq_t[:, 1, :],
                             start=False, stop=not with_bias)
            if with_bias:
                nc.tensor.matmul(qpsum, ones_t, bq_t, start=False, stop=True)
            qnat = sb_q.tile([128, 256], F16, tag="qnat")
            nc.vector.tensor_copy(out=qnat, in_=qpsum)
            nc.scalar.dma_start(
                out=qdram[mol].rearrange("(a c) -> a c", a=128), in_=qnat)

        qu_view = qdram[:].rearrange(
            "b (g p r k) -> b g p r k", g=2, p=128, r=4, k=32)

        for mol in range(bm):
            for G in range(2):
                # ---- X load (one converting DMA) + XBAR transpose ----
                xu16 = sb_xu.tile([128, 2560], F16, tag="xu")
                nc.gpsimd.dma_start(out=xu16, in_=xg[mol, G])
                xtf = sb_xtf.tile([128, 20, 128], F16, tag="xtf")
                nc.sync.dma_start_transpose(out=xtf, in_=xu16)
                if USE_FP8_K or USE_FP8_M:
                    xt8 = sb_xt8.tile([128, 20, 128], F8, tag="xt8")
                    nc.scalar.copy(out=xt8[:, 0:10, :], in_=xtf[:, 0:10, :])
                    nc.gpsimd.tensor_copy(out=xt8[:, 10:20, :],
                                          in_=xtf[:, 10:20, :])

                cur4 = sb_ew.tile([128, 4, 32], F32, tag="cur4")
                nc.sync.dma_start(out=cur4, in_=q5[mol, G])
                qu4 = sb_ew.tile([128, 4, 32], F16, tag="qu4")
                nc.sync.dma_start(out=qu4, in_=qu_view[mol, G])

                smulB = sb_ew.tile([128, 4, 320], F16, tag="smulB")
                v16B = sb_ew.tile([128, 4, 320], F16, tag="v16B")
                amulB = sb_ew.tile([128, 4, 32, 10], F16, tag="amulB")
                exB = sb_ew.tile([128, 4, 10], F16, tag="exB")
                scoreB = sb_ew.tile([128, 4, 10], F16, tag="scoreB")
                emaxB = sb_ew.tile([128, 4, 32], F16, tag="emaxB")
                adenB = sb_ew.tile([128, 4], F32, tag="adenB")
                arawB = sb_stash.tile([128, 4, 32], F16, tag="arawB")
                gaveB = sb_ew.tile([128, 4], F32, tag="gaveB")
                pg = ps_misc.tile([128, 4], F32, tag="pm", name="pg")

                for r in range(4):
                    wA = 256 - 64 * r
                    ranges = [(r, 0, wA, 64 * r), (r + 1, wA, 320 - wA, 0)]
                    kp = ps_k.tile([128, 320], F32, tag="pk", name=f"kp{r}",
                                   padded_shape=[128, 512])
                    vp = ps_v.tile([128, 320], F32, tag="pv", name=f"vp{r}",
                                   padded_shape=[128, 512])
                    mp = ps_m.tile([128, 320], F32, tag="pmx", name=f"mp{r}",
                                   padded_shape=[128, 512])
                    for (d, t0, wd, e0) in ranges:
                        # K and M (fp8 DoubleRow) with stationary xt8 pairs.
                        # K's start=True must be the first write into the kp
                        # bank: it marks the whole 2KB bank pending-zero,
                        # which also zero-initializes pg's column (pg then
                        # accumulates with start=False throughout).
                        for pair in range(2):
                            c0 = 4 * d + 2 * pair
                            if USE_FP8_K:
                                nc.tensor.matmul(
                                    kp[:, t0:t0 + wd],
                                    xt8[:, c0:c0 + 2, :],
                                    w8k_t[:, pair, :, e0:e0 + wd],
                                    start=(d == r and pair == 0),
                                    stop=(d == r + 1 and pair == 1
                                          and not with_bias),
                                    perf_mode=DR, skip_group_check=True)
                            if USE_FP8_M:
                                nc.tensor.matmul(
                                    mp[:, t0:t0 + wd],
                                    xt8[:, c0:c0 + 2, :],
                                    w8m_t[:, pair, :, e0:e0 + wd],
                                    start=(d == r and pair == 0),
                                    stop=(d == r + 1 and pair == 1
                                          and not with_bias),
                                    perf_mode=DR, skip_group_check=True)
                        if not USE_FP8_K or not USE_FP8_M:
                            for fc in range(4):
                                ch = 4 * d + fc
                                if not USE_FP8_K:
                                    nc.tensor.matmul(
                                        kp[:, t0:t0 + wd], xtf[:, ch, :],
                                        w16k_t[:, fc, e0:e0 + wd],
                                        start=(d == r and fc == 0),
                                        stop=(d == r + 1 and fc == 3
                                              and not with_bias),
                                        skip_group_check=True)
                                if not USE_FP8_M:
                                    nc.tensor.matmul(
                                        mp[:, t0:t0 + wd], xtf[:, ch, :],
                                        w16m_t[:, fc, e0:e0 + wd],
                                        start=(d == r and fc == 0),
                                        stop=(d == r + 1 and fc == 3
                                              and not with_bias),
                                        skip_group_check=True)
                        # V (f16) with stationary xtf chunks; pg rides along
                        # (start=False: K's bank-wide start zero-inits pg)
                        for fc in range(4):
                            ch = 4 * d + fc
                            nc.tensor.matmul(
                                vp[:, t0:t0 + wd], xtf[:, ch, :],
                                wv_t[:, fc, e0:e0 + wd],
                                start=(d == r and fc == 0),
                                stop=(d == r + 1 and fc == 3
                                      and not with_bias),
                                skip_group_check=True)
                            w_pg = ch - 5 * r
                            if 0 <= w_pg < 5:
                                nc.tensor.matmul(
                                    pg[:, r:r + 1], xtf[:, ch, :],
                                    wgav_t[:, w_pg:w_pg + 1],
                                    start=(w_pg == 0), stop=(w_pg == 4),
                                    skip_group_check=True)
                        if with_bias:
                            for ps, i in ((kp, 0), (vp, 1), (mp, 2)):
                                nc.tensor.matmul(
                                    ps[:, t0:t0 + wd], ones_t,
                                    bcat_t[:, i, e0:e0 + wd],
                                    start=False, stop=(d == r + 1),
                                    skip_group_check=True)

                    # ---- per-phase PSUM consumers (free the banks) ----
                    # V -> SBUF f16 on scalar (gpsimd cannot read PSUM)
                    nc.scalar.copy(out=v16B[:, r, :], in_=vp[:, 0:320])
                    # score product on vector (PSUM-sourced)
                    nc.vector.tensor_mul(
                        smulB[:, r, :], kp[:, 0:320],
                        qu4[:, r, :].unsqueeze(1).broadcast_to([128, 10, 32]))
                    nc.vector.tensor_reduce(
                        out=emaxB[:, r, :],
                        in_=mp[:, 0:320].rearrange("p (j k) -> p k j", j=10),
                        axis=AXL_X, op=MAX)

                # ---- batched per-G math (all SBUF f16) ----
                nc.vector.tensor_reduce(
                    out=scoreB,
                    in_=smulB.rearrange("p r (j k) -> p r j k", j=10),
                    axis=AXL_X, op=ADD)
                nc.scalar.activation(out=exB, in_=scoreB, func=EXP)
                nc.vector.tensor_reduce(out=adenB, in_=exB, axis=AXL_X, op=ADD)
                # attention numerator, written k-major for a contiguous
                # inner reduce
                nc.gpsimd.tensor_mul(
                    amulB.rearrange("p r k j -> p r j k"),
                    v16B.rearrange("p r (j k) -> p r j k", j=10),
                    exB.unsqueeze(3).broadcast_to([128, 4, 10, 32]))
                nc.vector.tensor_reduce(
                    out=arawB, in_=amulB, axis=AXL_X, op=ADD)
                nc.vector.tensor_copy(out=gaveB, in_=pg)
                curp = sb_ew.tile([128, 4, 32], F32, tag="curp")
                nc.gpsimd.tensor_mul(
                    curp, cur4,
                    wgc_t.unsqueeze(1).broadcast_to([128, 4, 32]))
                gcurB = sb_ew.tile([128, 4], F32, tag="gcurB")
                nc.vector.tensor_reduce(out=gcurB, in_=curp, axis=AXL_X,
                                        op=ADD)
                emaxp = sb_ew.tile([128, 4, 32], F16, tag="emaxp")
                nc.gpsimd.tensor_mul(
                    emaxp, emaxB,
                    wge_t.unsqueeze(1).broadcast_to([128, 4, 32]))
                gemxB = sb_ew.tile([128, 4], F32, tag="gemxB")
                nc.vector.tensor_reduce(out=gemxB, in_=emaxp, axis=AXL_X,
                                        op=ADD)
                gl1 = sb_ew.tile([128, 4], F32, tag="gl1")
                nc.vector.tensor_add(gl1, gcurB, gemxB)
                gl2 = sb_ew.tile([128, 4], F32, tag="gl2")
                nc.vector.tensor_add(gl2, gl1, gaveB)
                egB = sb_stash.tile([128, 4], F32, tag="egB")
                nc.scalar.activation(out=egB, in_=gl2, func=EXP,
                                     bias=float(bg_val))
                egB16 = sb_stash.tile([128, 4], F16, tag="egB16")
                nc.vector.tensor_copy(out=egB16, in_=egB)
                raB = sb_stash.tile([128, 4], F32, tag="raB")
                nc.vector.reciprocal(out=raB, in_=adenB)

                if G == 0:
                    st0 = (arawB, egB, egB16, raB)
                else:
                    gd = ps_misc.tile([32, 4], F32, tag="pm", name="gd")
                    for r in range(4):
                        nc.tensor.matmul(gd[:, r:r + 1], ssel_t,
                                         st0[2][:, r:r + 1],
                                         start=True, stop=False)
                        nc.tensor.matmul(gd[:, r:r + 1], ssel_t,
                                         egB16[:, r:r + 1],
                                         start=False, stop=True)
                    rg = sb_ew.tile([32, 4], F32, tag="rg")
                    nc.vector.reciprocal(out=rg, in_=gd)
                    rg16 = sb_ew.tile([32, 4], F16, tag="rg16")
                    nc.vector.tensor_copy(out=rg16, in_=rg)
                    inv = ps_misc.tile([128, 4], F32, tag="pm", name="inv")
                    for r in range(4):
                        nc.tensor.matmul(inv[:, r:r + 1], s2sel_t,
                                         rg16[:, r:r + 1],
                                         start=True, stop=True)
                    c2B = {}
                    for gg, (ar_g, eg_g, eg16_g, ra_g) in (
                            (0, st0), (1, (arawB, egB, egB16, raB))):
                        t1 = sb_ew.tile([128, 4], F32, tag="t1", name="t1")
                        nc.vector.tensor_mul(t1, inv, ra_g)
                        c2B[gg] = sb_stash.tile([128, 4], F32, tag=f"c2B{gg}",
                                                name=f"c2B{gg}")
                        nc.vector.tensor_mul(c2B[gg], t1, eg_g)
                    for gg, ar_g in ((0, st0[0]), (1, arawB)):
                        outB = sb_ew.tile([128, 4, 32], F32, tag="outB")
                        nc.gpsimd.tensor_mul(
                            outB, ar_g,
                            c2B[gg].unsqueeze(2).broadcast_to([128, 4, 32]))
                        nc.scalar.dma_start(out=o5[mol, gg], in_=outB)
    nc.finalize()
    return nc


def _q8(w):
    import ml_dtypes
    return np.asarray(w, np.float32).astype(ml_dtypes.float8_e4m3fn)


def _prep_consts(Wq, bq, Wk, bk, Wv, bv, Wam, bam, Wg, bg):
    wv = np.empty((128, 4, 256), np.float16)
    for fc in range(4):
        wv[:, fc, :] = Wv[128 * fc:128 * (fc + 1), :]
    consts = {"wv": wv}
    kscale = WSCALE if USE_FP8_K else 1.0
    mscale = WSCALE if USE_FP8_M else 1.0
    if USE_FP8_K:
        w8k = np.empty((128, 2, 2, 256), np.float32)
        for pair in range(2):
            for i in range(2):
                fc = 2 * pair + i
                w8k[:, pair, i, :] = Wk[128 * fc:128 * (fc + 1), :] * kscale
        consts["w8k"] = _q8(w8k)
    else:
        w16k = np.empty((128, 4, 256), np.float16)
        for fc in range(4):
            w16k[:, fc, :] = Wk[128 * fc:128 * (fc + 1), :]
        consts["w16k"] = w16k
    if USE_FP8_M:
        w8m = np.empty((128, 2, 2, 256), np.float32)
        for pair in range(2):
            for i in range(2):
                fc = 2 * pair + i
                w8m[:, pair, i, :] = Wam[128 * fc:128 * (fc + 1), :] * mscale
        consts["w8m"] = _q8(w8m)
    else:
        w16m = np.empty((128, 4, 256), np.float16)
        for fc in range(4):
            w16m[:, fc, :] = Wam[128 * fc:128 * (fc + 1), :]
        consts["w16m"] = w16m
    wqs = np.empty((128, 2, 256), np.float16)
    for fc in range(2):
        wqs[:, fc, :] = Wq[128 * fc:128 * (fc + 1), :] / kscale
    consts["wq"] = wqs
    p = np.arange(128)
    ssel = (p[:, None] % 32 == np.arange(32)[None, :]).astype(np.float16)
    consts["ssel"] = ssel
    consts["s2sel"] = ssel.T.copy()
    wg = np.asarray(Wg[:, 0], np.float32)
    consts["wg_cur"] = np.tile(wg[0:32], (128, 1)).astype(np.float32)
    consts["wg_emax"] = np.tile(wg[32:64] / mscale, (128, 1)).astype(np.float16)
    wgav = np.empty((128, 5), np.float32)
    for w in range(5):
        wgav[:, w] = wg[64 + (np.arange(128) % 64)] / NEI
    consts["wg_avc"] = wgav.astype(np.float16)
    with_bias = any(np.any(np.asarray(b) != 0) for b in (bq, bk, bv, bam))
    if with_bias:
        bcat = np.stack([np.asarray(bk) * kscale, np.asarray(bv),
                         np.asarray(bam) * mscale]
                        ).astype(np.float16)[None, :, :].reshape(1, 3, 256)
        consts["bcat"] = bcat
        consts["bq"] = (np.asarray(bq) / kscale).astype(np.float16).reshape(
            1, 256)
        consts["ones"] = np.ones((1, 128), np.float16)
    return consts, with_bias, float(np.asarray(bg).reshape(-1)[0])


_CACHE = {}
TRACE = False       # set by test.py for profiling runs
LAST_RESULTS = None  # BassKernelResults from the most recent run


def kernel(input_multihead, input_q, Wq, bq, Wk, bk, Wv, bv, Wam, bam, Wg, bg):
    from concourse.bass_utils import run_bass_kernel_spmd

    consts, with_bias, bg_val = _prep_consts(
        Wq, bq, Wk, bk, Wv, bv, Wam, bam, Wg, bg)

    key = (with_bias, bg_val)
    if key not in _CACHE:
        _CACHE[key] = build_nc(with_bias, bg_val)
    nc = _CACHE[key]

    x = np.ascontiguousarray(np.asarray(input_multihead, np.float32))
    q = np.ascontiguousarray(np.asarray(input_q, np.float32))
    in_maps = []
    for c in range(N_CORES):
        m = {"x": x[BM * c:BM * (c + 1)], "qin": q[BM * c:BM * (c + 1)]}
        m.update(consts)
        in_maps.append(m)

    res = run_bass_kernel_spmd(nc, in_maps, list(range(N_CORES)), trace=TRACE)
    global LAST_RESULTS
    LAST_RESULTS = res
    return np.concatenate([res.results[c]["out"] for c in range(N_CORES)],
                          axis=0)
